# revision 31
# baseline (speedup 1.0000x reference)
"""MLA attention (DeepSeek-style, LoRA Q/KV) on 8 Trainium2 NeuronCores.

Sharding (two SPMD launches, all tensor math in fp16 with fp32 PSUM):
  L1 (sequence-parallel, 256 tokens/core, token-major): each core computes
      t_raw  = x @ Wqa   -> rmsnorm -> tn  [256, 1536]
      ckv    = x @ Wkva  -> comp rmsnorm / kpe rope -> ckv [256, 576]
  Token-major layout makes the rmsnorm a per-partition (per-token) scale,
  applied by the Activation engine during the PSUM->SBUF copy; the sum of
  squares comes from an ACT Square pass with accum_out.  Weights are the
  moving matmul operand so the k-chunked DMA stream overlaps compute.
  Host gathers along tokens and transposes to feature-major (cheap numpy),
  then
  L2 (tensor-parallel, 2 heads/core): q/k/v LoRA-B projections, rope(q),
  scores^T = k @ q^T, biased exp (exp(s*scale - 2), bias cancels in the
  softmax ratio), softmax denominator via sequential fp16 adds of the exp
  tiles on the Vector engine + one Pool-engine partition_all_reduce (no
  TensorE work), attn_out^T = v @ exp^T, per-head normalize, output
  projection with this core's Wo row-slice.  Host sums the 8 partials.
"""

import math
from contextlib import ExitStack

import numpy as np

import concourse.mybir as mybir
import concourse.tile as tile
from concourse import bacc, bass_isa
from concourse.bass_utils import run_bass_kernel_spmd

F16 = mybir.dt.float16
F32 = mybir.dt.float32
AF = mybir.ActivationFunctionType
NF16 = np.float16

D_MODEL = 2048
NH = 16
Q_LORA = 1536
KV_LORA = 512
ROPE = 64
NOPE = 128
VDIM = 128
QHD = NOPE + ROPE  # 192
SEQ = 2048
N_CORES = 8
S_LOC = SEQ // N_CORES  # 256 tokens per core in L1
EPS = 1e-6
SCALE = 1.0 / math.sqrt(128.0)  # 1/sqrt(HEAD_DIM), as in the reference
EXP_BIAS = -2.0  # exp(s*SCALE + b): constant bias cancels in softmax

_CACHE = {}


def _perm_rope_T(n):
    """lhsT for P @ v where (P@v)[2i] = -v[2i+1], (P@v)[2i+1] = v[2i]."""
    P = np.zeros((n, n), np.float32)
    for i in range(n // 2):
        P[2 * i, 2 * i + 1] = -1.0
        P[2 * i + 1, 2 * i] = 1.0
    return np.ascontiguousarray(P.T).astype(NF16)


# --------------------------------------------------------------------------
# Launch 1: sequence-sharded LoRA-A projections + norms + k_pe rope
# (token-major: out[token, feature], weights are the moving operand)
# --------------------------------------------------------------------------

def build_l1():
    nc = bacc.Bacc("TRN2", target_bir_lowering=False, debug=False,
                   enable_asserts=True, num_devices=N_CORES)
    KD = D_MODEL // 128   # 16 contraction chunks

    xT = nc.dram_tensor("xT", [D_MODEL, S_LOC], F16, kind="ExternalInput").ap()
    Wqa = nc.dram_tensor("Wqa", [D_MODEL, Q_LORA], F16, kind="ExternalInput").ap()
    Wkva = nc.dram_tensor("Wkva", [D_MODEL, 576], F16, kind="ExternalInput").ap()
    # token-major cos/sin, pairs repeated: [S_LOC, 64] f32
    cosW = nc.dram_tensor("cosW", [S_LOC, ROPE], F32, kind="ExternalInput").ap()
    sinW = nc.dram_tensor("sinW", [S_LOC, ROPE], F32, kind="ExternalInput").ap()

    tn = nc.dram_tensor("tn", [S_LOC, Q_LORA], F16, kind="ExternalOutput").ap()
    ckv = nc.dram_tensor("ckv", [S_LOC, 576], F16, kind="ExternalOutput").ap()
    rq = nc.dram_tensor("rq", [S_LOC], F32, kind="ExternalOutput").ap()
    rkv = nc.dram_tensor("rkv", [S_LOC], F32, kind="ExternalOutput").ap()

    with tile.TileContext(nc) as tc, ExitStack() as ctx:
        const = ctx.enter_context(tc.tile_pool(name="const", bufs=1))
        big = ctx.enter_context(tc.tile_pool(name="big", bufs=1))
        work = ctx.enter_context(tc.tile_pool(name="work", bufs=2))
        ps = ctx.enter_context(tc.tile_pool(name="ps", bufs=1, space="PSUM"))

        # ---- DMA, one consumption-ordered queue: xT and Wqa column-0
        # k-chunks interleaved (Q phase runs first), then Wqa c1/c2, then
        # Wkva for the trailing KV phase.
        xT_r = xT.rearrange("(k p) s -> p k s", p=128)
        wqa_r = Wqa.rearrange("(k p) l -> p k l", p=128)
        wkva_r = Wkva.rearrange("(k p) l -> p k l", p=128)
        sb_xT = big.tile([128, KD, S_LOC], F16, tag="xT")
        sb_wqa = big.tile([128, KD, Q_LORA], F16, tag="wqa")
        sb_wkva = big.tile([128, KD, 576], F16, tag="wkva")
        for kg in range(0, KD, 4):
            nc.sync.dma_start(sb_xT[:, kg:kg + 4, :], xT_r[:, kg:kg + 4, :])
            nc.sync.dma_start(sb_wqa[:, kg:kg + 4, 0:512],
                              wqa_r[:, kg:kg + 4, 0:512])
        for kg in range(0, KD, 4):
            nc.sync.dma_start(sb_wqa[:, kg:kg + 4, 512:1024],
                              wqa_r[:, kg:kg + 4, 512:1024])
        for kg in range(0, KD, 8):
            nc.sync.dma_start(sb_wkva[:, kg:kg + 8, :], wkva_r[:, kg:kg + 8, :])
        for kg in range(0, KD, 4):
            nc.sync.dma_start(sb_wqa[:, kg:kg + 4, 1024:1536],
                              wqa_r[:, kg:kg + 4, 1024:1536])
        cos_r = cosW.rearrange("(t p) c -> p t c", p=128)
        sin_r = sinW.rearrange("(t p) c -> p t c", p=128)
        sb_cos = const.tile([128, 2, ROPE], F32, tag="cos")
        nc.scalar.dma_start(sb_cos[:], cos_r)
        sb_sin = const.tile([128, 2, ROPE], F32, tag="sin")
        nc.scalar.dma_start(sb_sin[:], sin_r)

        eps_t = const.tile([128, 1], F32, tag="eps")
        nc.vector.memset(eps_t[:], EPS)
        tn_r = tn.rearrange("(t p) f -> p t f", p=128)
        ckv_r = ckv.rearrange("(t p) f -> p t f", p=128)
        rq_r = rq.rearrange("(t p) -> p t", p=128)
        rkv_r = rkv.rearrange("(t p) -> p t", p=128)

        sq_dump = work.tile([128, 3 * 512], F16, tag="sqdump", bufs=2)

        def rms_r(sq_ap, dim, tag):
            """r = 1/sqrt(mean(sq)+eps) per partition (token)."""
            rstd = work.tile([128, 1], F32, tag=tag + "rs")
            nc.scalar.activation(rstd[:], sq_ap, AF.Sqrt, bias=eps_t[:],
                                 scale=1.0 / dim)
            r = work.tile([128, 1], F32, tag=tag + "r")
            nc.vector.reciprocal(r[:], rstd[:])
            return r

        # ---- Q phase (cols 0,1), then KV, then Q col 2.  Raw fp16
        # activations stream out as each chunk lands; only the tiny 1/rms
        # scale is computed here (normalization is folded into L2's copies).
        q_raw = big.tile([128, 2, 3 * 512], F16, tag="qraw")

        q_sq = [[None] * 3 for _ in range(2)]

        def q_tail(T):
            acc = q_sq[T][0][:]
            for c in (1, 2):
                nc.vector.tensor_add(acc, acc, q_sq[T][c][:])
            r = rms_r(acc, Q_LORA, f"q{T}")
            nc.sync.dma_start(rq_r[:, T:T + 1], r[:])

        q_cols = (0, 1)
        for c in q_cols:
            for T in range(2):
                acc = ps.tile([128, 512], F32, tag="q", bufs=4)
                for k in range(KD):
                    nc.tensor.matmul(acc[:], sb_xT[:, k, T * 128:(T + 1) * 128],
                                     sb_wqa[:, k, c * 512:(c + 1) * 512],
                                     start=(k == 0), stop=(k == KD - 1))
                nc.scalar.copy(q_raw[:, T, c * 512:(c + 1) * 512], acc[:])
                nc.sync.dma_start(tn_r[:, T, c * 512:(c + 1) * 512],
                                  q_raw[:, T, c * 512:(c + 1) * 512])
                sq = work.tile([128, 1], F32, tag=f"qsq{T}{c}")
                nc.scalar.activation(sq_dump[:, 0:512],
                                     q_raw[:, T, c * 512:(c + 1) * 512],
                                     AF.Square, accum_out=sq[:])
                q_sq[T][c] = sq
                if c == 2:
                    q_tail(T)


        # ---- KV phase (sandwiched between Q columns; tails hide under
        # the final Q column)
        ckv_sb = big.tile([128, 2, 576], F16, tag="ckvsb")
        for T in range(2):
            acc = ps.tile([128, 576], F32, tag="kv", bufs=2)
            for k in range(KD):
                nc.tensor.matmul(acc[:, 0:512], sb_xT[:, k, T * 128:(T + 1) * 128],
                                 sb_wkva[:, k, 0:512],
                                 start=(k == 0), stop=(k == KD - 1))
                nc.tensor.matmul(acc[:, 512:576], sb_xT[:, k, T * 128:(T + 1) * 128],
                                 sb_wkva[:, k, 512:576],
                                 start=(k == 0), stop=(k == KD - 1))
            nc.scalar.copy(ckv_sb[:, T, 0:512], acc[:, 0:512])
            sq = work.tile([128, 1], F32, tag="kvsq", bufs=2)
            nc.scalar.activation(sq_dump[:, 0:512], ckv_sb[:, T, 0:512],
                                 AF.Square, accum_out=sq[:])
            r = rms_r(sq[:], KV_LORA, f"kv{T}")
            nc.sync.dma_start(rkv_r[:, T:T + 1], r[:])
            # kpe rope: free-dim pair swap + cos/sin combine (DVE, tiny)
            sw = work.tile([128, ROPE], F32, tag="sw", bufs=2)
            nc.vector.tensor_scalar_mul(sw[:, 0:ROPE:2], acc[:, 513:576:2], -1.0)
            nc.vector.tensor_copy(sw[:, 1:ROPE:2], acc[:, 512:576:2])
            m1 = work.tile([128, ROPE], F32, tag="m1", bufs=2)
            nc.vector.tensor_mul(m1[:], acc[:, 512:576], sb_cos[:, T, :])
            m2 = work.tile([128, ROPE], F32, tag="m2", bufs=2)
            nc.vector.tensor_mul(m2[:], sw[:], sb_sin[:, T, :])
            nc.vector.tensor_add(ckv_sb[:, T, 512:576], m1[:], m2[:])
            nc.sync.dma_start(ckv_r[:, T, :], ckv_sb[:, T, :])

        q_cols = (2,)
        for c in q_cols:
            for T in range(2):
                acc = ps.tile([128, 512], F32, tag="q", bufs=4)
                for k in range(KD):
                    nc.tensor.matmul(acc[:], sb_xT[:, k, T * 128:(T + 1) * 128],
                                     sb_wqa[:, k, c * 512:(c + 1) * 512],
                                     start=(k == 0), stop=(k == KD - 1))
                nc.scalar.copy(q_raw[:, T, c * 512:(c + 1) * 512], acc[:])
                nc.sync.dma_start(tn_r[:, T, c * 512:(c + 1) * 512],
                                  q_raw[:, T, c * 512:(c + 1) * 512])
                sq = work.tile([128, 1], F32, tag=f"qsq{T}{c}")
                nc.scalar.activation(sq_dump[:, 0:512],
                                     q_raw[:, T, c * 512:(c + 1) * 512],
                                     AF.Square, accum_out=sq[:])
                q_sq[T][c] = sq
                if c == 2:
                    q_tail(T)

    nc.compile()
    return nc


# --------------------------------------------------------------------------
# Launch 2: head-sharded attention (2 heads per core)
# --------------------------------------------------------------------------

def build_l2():
    nc = bacc.Bacc("TRN2", target_bir_lowering=False, debug=False,
                   enable_asserts=True, num_devices=N_CORES)
    KQ = Q_LORA // 128    # 12
    KKV = KV_LORA // 128  # 4
    ST = SEQ // 128       # 16 key tiles
    SB = 1024             # query block
    NSB = SEQ // SB       # 2
    PRE_V = 2             # v-groups emitted right after kn (pre-paced)
    FILL_SCHED = [2, 1, 1, 1, 1, 0]  # v-groups per tnT k-pair in paced q

    tnT = nc.dram_tensor("tnT", [Q_LORA, SEQ], F16, kind="ExternalInput").ap()
    compT = nc.dram_tensor("compT", [KV_LORA, SEQ], F16, kind="ExternalInput").ap()
    kpeT = nc.dram_tensor("kpeT", [ROPE, SEQ], F16, kind="ExternalInput").ap()
    # Wqb cols reordered [h0 nope | h1 nope | h0 rope | h1 rope], qln folded
    Wqb = nc.dram_tensor("Wqb", [Q_LORA, 2 * QHD], F16, kind="ExternalInput").ap()
    Wkn = nc.dram_tensor("Wkn", [KV_LORA, 2 * NOPE], F16, kind="ExternalInput").ap()
    Wv = nc.dram_tensor("Wv", [KV_LORA, 2 * VDIM], F16, kind="ExternalInput").ap()
    Wo = nc.dram_tensor("Wo", [2 * VDIM, D_MODEL], F16, kind="ExternalInput").ap()
    rqv = nc.dram_tensor("rqv", [SEQ], F16, kind="ExternalInput").ap()
    rkvv = nc.dram_tensor("rkvv", [SEQ], F16, kind="ExternalInput").ap()
    rkv32 = nc.dram_tensor("rkv32", [SEQ], F32, kind="ExternalInput").ap()
    cosT2 = nc.dram_tensor("cosT2", [128, SEQ], F16, kind="ExternalInput").ap()
    sinT2 = nc.dram_tensor("sinT2", [128, SEQ], F16, kind="ExternalInput").ap()
    permT2 = nc.dram_tensor("permT2", [128, 128], F16, kind="ExternalInput").ap()

    out = nc.dram_tensor("out", [SEQ, D_MODEL], F16, kind="ExternalOutput").ap()

    with tile.TileContext(nc) as tc, ExitStack() as ctx:
        const = ctx.enter_context(tc.tile_pool(name="const", bufs=1))
        big = ctx.enter_context(tc.tile_pool(name="big", bufs=1))
        tmp1 = ctx.enter_context(tc.tile_pool(name="tmp1", bufs=1))
        work = ctx.enter_context(tc.tile_pool(name="work", bufs=2))
        exp_pool = ctx.enter_context(tc.tile_pool(name="expp", bufs=2))
        psum = ctx.enter_context(tc.tile_pool(name="psum", bufs=1, space="PSUM"))

        # DMA in consumption order.  Small weights on the ACT HWDGE queue,
        # big activation streams on the SP queue.
        sb_wkn = big.tile([128, KKV, 2 * NOPE], F16, tag="wkn")
        nc.sync.dma_start(sb_wkn[:], Wkn.rearrange("(k p) n -> p k n", p=128))
        sb_rkv1 = const.tile([1, SEQ], F16, tag="rkv1")
        nc.scalar.dma_start(sb_rkv1[:], rkvv.rearrange("(o s) -> o s", o=1))
        sb_rq1 = const.tile([1, SEQ], F16, tag="rq1")
        nc.scalar.dma_start(sb_rq1[:], rqv.rearrange("(o s) -> o s", o=1))
        # token-tiled layout for the per-partition v scaling (ACT needs f32)
        sb_rkv2 = const.tile([128, ST], F32, tag="rkv2")
        nc.scalar.dma_start(sb_rkv2[:], rkv32.rearrange("(t p) -> p t", p=128))
        rkv_bc = big.tile([128, SEQ], F16, tag="rkvbc")
        nc.gpsimd.partition_broadcast(rkv_bc[:], sb_rkv1[:])
        rq_bc = big.tile([128, SEQ], F16, tag="rqbc")
        nc.gpsimd.partition_broadcast(rq_bc[:], sb_rq1[:])
        sb_compT = big.tile([128, KKV, SEQ], F16, tag="compT")
        compT_r = compT.rearrange("(k p) s -> p k s", p=128)
        # column-major chunks so kn's n-th group only waits on chunk n
        for n in range(4):
            nc.sync.dma_start(sb_compT[:, :, n * 512:(n + 1) * 512],
                              compT_r[:, :, n * 512:(n + 1) * 512])
        sb_wv = big.tile([128, KKV, 2 * VDIM], F16, tag="wv")
        nc.sync.dma_start(sb_wv[:], Wv.rearrange("(k p) n -> p k n", p=128))
        sb_wqb = big.tile([128, KQ, 2 * QHD], F16, tag="wqb")
        nc.sync.dma_start(sb_wqb[:], Wqb.rearrange("(k p) n -> p k n", p=128))
        sb_tnT = big.tile([128, KQ, SEQ], F16, tag="tnT")
        tnT_r = tnT.rearrange("(k p) s -> p k s", p=128)
        sb_perm2 = const.tile([128, 128], F16, tag="perm2")
        sb_cos2 = const.tile([128, SEQ], F16, tag="cos2")
        sb_sin2 = const.tile([128, SEQ], F16, tag="sin2")
        # one strictly consumption-ordered queue: tnT chunks interleave with
        # the rope constants so both are resident when first needed
        nc.sync.dma_start(sb_tnT[:, 0:2, :], tnT_r[:, 0:2, :])
        nc.sync.dma_start(sb_perm2[:], permT2)
        nc.sync.dma_start(sb_tnT[:, 2:4, :], tnT_r[:, 2:4, :])
        nc.sync.dma_start(sb_cos2[:], cosT2)
        nc.sync.dma_start(sb_tnT[:, 4:6, :], tnT_r[:, 4:6, :])
        nc.sync.dma_start(sb_sin2[:], sinT2)
        for k in range(6, KQ, 2):
            nc.sync.dma_start(sb_tnT[:, k:k + 2, :], tnT_r[:, k:k + 2, :])
        sb_kpe = big.tile([ROPE, SEQ], F16, tag="kpe")
        nc.sync.dma_start(sb_kpe[:], kpeT)
        sb_wo = big.tile([128, 2, D_MODEL], F16, tag="wo")
        nc.sync.dma_start(sb_wo[:], Wo.rearrange("(k p) n -> p k n", p=128))

        ebias_t = const.tile([128, 1], F32, tag="ebias")
        nc.vector.memset(ebias_t[:], EXP_BIAS)

        # ---- k_nope^T per head, n-major so it is paced by the compT stream
        k_nope = [big.tile([128, SEQ], F16, tag=f"kn{h}", name=f"kn{h}")
                  for h in range(2)]
        for n in range(SEQ // 512):
            for h in range(2):
                acc = psum.tile([128, 512], F32, tag="sc", bufs=4, name="kacc")
                for k in range(KKV):
                    nc.tensor.matmul(acc[:], sb_wkn[:, k, h * 128:(h + 1) * 128],
                                     sb_compT[:, k, n * 512:(n + 1) * 512],
                                     start=(k == 0), stop=(k == KKV - 1))
                nc.vector.tensor_mul(k_nope[h][:, n * 512:(n + 1) * 512],
                                     acc[:], rkv_bc[:, n * 512:(n + 1) * 512])

        # ---- v natural [key, vd] per head, 4 key-tiles per PSUM tile.
        # Emitted as filler thunks inside the tnT-DMA-paced part of the q
        # projection (v depends only on compT + Wv, which land early).
        v_nat = [big.tile([128, ST, VDIM], F16, tag=f"v{h}", name=f"vn{h}")
                 for h in range(2)]

        def v_group(h, g):
            def go():
                acc = psum.tile([128, 512], F32, tag="sc", bufs=4, name="vacc")
                for i in range(4):
                    t = g * 4 + i
                    for k in range(KKV):
                        nc.tensor.matmul(acc[:, i * 128:(i + 1) * 128],
                                         sb_compT[:, k, t * 128:(t + 1) * 128],
                                         sb_wv[:, k, h * VDIM:(h + 1) * VDIM],
                                         start=(k == 0), stop=(k == KKV - 1))
                for i in range(4):
                    t = g * 4 + i
                    if i % 2 == 0:
                        nc.scalar.activation(v_nat[h][:, t, :],
                                             acc[:, i * 128:(i + 1) * 128],
                                             AF.Copy, scale=sb_rkv2[:, t:t + 1])
                    else:
                        nc.vector.tensor_scalar_mul(v_nat[h][:, t, :],
                                                    acc[:, i * 128:(i + 1) * 128],
                                                    sb_rkv2[:, t:t + 1])
            return go

        fillers = [v_group(h, g) for g in range(ST // 4) for h in range(2)]

        # ---- q^T = Wqb^T @ tn^T : m-tiles [h0 nope, h1 nope, (h0|h1) rope]
        q_nope = [big.tile([128, SEQ], F16, tag=f"qn{h}", name=f"qn{h}")
                  for h in range(2)]
        qpe_raw = tmp1.tile([128, SEQ], F16, tag="qpe_raw")
        qpe2 = big.tile([128, SEQ], F16, tag="qpe2")
        qswap = tmp1.tile([128, SEQ], F16, tag="qswap")

        def q_dst(m):
            return q_nope[m] if m < 2 else qpe_raw

        def rope_chunk(n):
            sl = slice(n * 512, (n + 1) * 512)
            sw = psum.tile([128, 512], F32, tag="sc", bufs=4, name="swps")
            nc.tensor.matmul(sw[:], sb_perm2[:], qpe_raw[:, sl],
                             start=True, stop=True)
            nc.scalar.copy(qswap[:, sl], sw[:])
            nc.vector.tensor_mul(qpe2[:, sl], qpe_raw[:, sl], sb_cos2[:, sl])
            nc.vector.tensor_mul(qswap[:, sl], qswap[:, sl], sb_sin2[:, sl])
            nc.vector.tensor_add(qpe2[:, sl], qpe2[:, sl], qswap[:, sl])

        # every column is a k-pair-interleaved m-triple so PE blocks only on
        # the genuinely-missing tnT chunk; v-group fillers absorb the rest of
        # the DMA-paced slack in the first wave
        for _ in range(PRE_V):
            fillers.pop(0)()
        for n in range(SEQ // 512):
            sl = slice(n * 512, (n + 1) * 512)
            accs = [psum.tile([128, 512], F32, tag=("sc" if m < 2 else "av"),
                              bufs=(4 if m < 2 else 2), name=f"qacc{n}{m}")
                    for m in range(3)]
            for kp in range(KQ // 2):
                nf = FILL_SCHED[kp] if n == 0 else 0
                per_m = [nf - 2 * (nf // 3), nf // 3, nf // 3]
                for m in range(3):
                    for k in (2 * kp, 2 * kp + 1):
                        nc.tensor.matmul(accs[m][:],
                                         sb_wqb[:, k, m * 128:(m + 1) * 128],
                                         sb_tnT[:, k, sl],
                                         start=(k == 0), stop=(k == KQ - 1))
                    for _ in range(per_m[m]):
                        if fillers:
                            fillers.pop(0)()
                            break
            for m in range(2):
                nc.vector.tensor_mul(q_dst(m)[:, sl], accs[m][:], rq_bc[:, sl])
            nc.scalar.copy(qpe_raw[:, sl], accs[2][:])
            if n > 0:
                rope_chunk(n - 1)
            if n == 0:
                while fillers:
                    fillers.pop(0)()
        rope_chunk(SEQ // 512 - 1)

        # h1 rope rows to a base-0 tile so matmul operands stay aligned
        qpe_h1 = big.tile([ROPE, SEQ], F16, tag="qpeh1")
        nc.gpsimd.dma_start(qpe_h1[:], qpe2[ROPE:128, :])

        def qpe_of(h):
            return qpe2[0:ROPE, :] if h == 0 else qpe_h1[:, :]

        # ---- attention per query block of SB, per head (exp streamed per
        # 512-wide chunk; denominator on DVE+Pool, no TensorE work)
        def attention_pass(sb_i, h):
            s0 = sb_i * SB
            av_ps = psum.tile([128, SB], F32, tag="av", bufs=2, name="av_ps")
            e_acc = work.tile([128, SB], F16, tag="eacc", bufs=2)

            def av_mm(t, n2, e):
                nc.tensor.matmul(av_ps[:, n2 * 512:(n2 + 1) * 512],
                                 v_nat[h][:, t, :], e[:],
                                 start=(t == 0), stop=(t == ST - 1))

            pending = []
            for t in range(ST):
                for n2 in range(2):
                    sl = slice(s0 + n2 * 512, s0 + (n2 + 1) * 512)
                    psl = slice(n2 * 512, (n2 + 1) * 512)
                    sc = psum.tile([128, 512], F32, tag="sc", bufs=4, name="sc")
                    nc.tensor.matmul(sc[:],
                                     k_nope[h][:, t * 128:(t + 1) * 128],
                                     q_nope[h][:, sl], start=True, stop=False)
                    nc.tensor.matmul(sc[:],
                                     sb_kpe[:, t * 128:(t + 1) * 128],
                                     qpe_of(h)[:, sl],
                                     start=False, stop=True)
                    expT = exp_pool.tile([128, 512], F16, tag="expT", bufs=6)
                    nc.scalar.activation(expT[:], sc[:], AF.Exp, bias=ebias_t[:],
                                         scale=SCALE)
                    if t == 0:
                        nc.vector.tensor_copy(e_acc[:, psl], expT[:])
                    else:
                        nc.vector.tensor_add(e_acc[:, psl], e_acc[:, psl], expT[:])
                    pending.append((t, n2, expT))
                    if len(pending) > 3:
                        av_mm(*pending.pop(0))
            for p_ in pending:
                av_mm(*p_)
            # denominator: one Pool partition_all_reduce, then per-chunk
            # reciprocal+scale so oproj can start on the first chunk early
            den_b = work.tile([128, SB], F32, tag="denb", bufs=2)
            nc.gpsimd.partition_all_reduce(den_b[:], e_acc[:], 128,
                                           bass_isa.ReduceOp.add)
            att = work.tile([128, SB], F16, tag=f"att{h}", name=f"att{h}")
            for n2 in range(2):
                psl = slice(n2 * 512, (n2 + 1) * 512)
                den_r = work.tile([128, 512], F32, tag="denr", bufs=2)
                nc.vector.reciprocal(den_r[:], den_b[:, psl])
                nc.vector.tensor_mul(att[:, psl], av_ps[:, psl], den_r[:])
            return att

        def oproj(sb_i, att_n, last=False):
            s0 = sb_i * SB
            for ms in range(SB // 128):
                o = work.tile([128, D_MODEL], F16, tag="osb", bufs=3)
                rows = slice(s0 + ms * 128, s0 + (ms + 1) * 128)
                chunked = last and ms == SB // 128 - 1
                for n in range(D_MODEL // 512):
                    cols = slice(n * 512, (n + 1) * 512)
                    acc = psum.tile([128, 512], F32, tag="sc", bufs=4,
                                    name="oacc")
                    for h in range(2):
                        nc.tensor.matmul(acc[:],
                                         att_n[h][:, ms * 128:(ms + 1) * 128],
                                         sb_wo[:, h, n * 512:(n + 1) * 512],
                                         start=(h == 0), stop=(h == 1))
                    if n % 2 == 0:
                        nc.scalar.copy(o[:, cols], acc[:])
                    else:
                        nc.vector.tensor_copy(o[:, cols], acc[:])
                    if chunked and n % 2 == 1:
                        half = slice((n - 1) * 512, (n + 1) * 512)
                        nc.sync.dma_start(out[rows, half], o[:, half])
                if not chunked:
                    nc.sync.dma_start(out[rows, :], o[:])

        # all passes first, then both output projections: the last
        # normalize chain hides under O0's matmuls instead of stalling PE
        a00 = attention_pass(0, 0)
        a01 = attention_pass(0, 1)
        a10 = attention_pass(1, 0)
        a11 = attention_pass(1, 1)
        oproj(0, [a00, a01])
        oproj(1, [a10, a11], last=True)

    nc.compile()
    return nc


# --------------------------------------------------------------------------
# Host orchestration
# --------------------------------------------------------------------------

def _prep(x, freqs_cis, Wqa, qln, Wqb, Wkva, kvln, Wkvb, Wo):
    """Host-side sharding prep (cheap numpy reshapes/casts only)."""
    xT = np.ascontiguousarray(x[0].T).astype(NF16)           # [D, S]
    cos = freqs_cis[..., 0].astype(np.float32)               # [S, 32]
    sin = freqs_cis[..., 1].astype(np.float32)
    cosW = np.repeat(cos, 2, axis=1)                         # [S, 64] tok-major
    sinW = np.repeat(sin, 2, axis=1)
    cosT = np.ascontiguousarray(cosW.T)                      # [64, S]
    sinT = np.ascontiguousarray(sinW.T)

    Wqb_f = Wqb * qln[:, None]
    Wkvb_f = Wkvb * kvln[:, None]
    Wqb_hd = Wqb_f.reshape(Q_LORA, NH, QHD)
    Wkvb_hd = Wkvb_f.reshape(KV_LORA, NH, NOPE + VDIM)
    Wo_hd = Wo.reshape(NH, VDIM, D_MODEL)
    l2_per_core = []
    for c in range(N_CORES):
        hs = [2 * c, 2 * c + 1]
        wqb_c = np.concatenate(
            [Wqb_hd[:, hs[0], :NOPE], Wqb_hd[:, hs[1], :NOPE],
             Wqb_hd[:, hs[0], NOPE:], Wqb_hd[:, hs[1], NOPE:]], axis=1)
        wkn_c = np.concatenate([Wkvb_hd[:, h, :NOPE] for h in hs], axis=1)
        wv_c = np.concatenate([Wkvb_hd[:, h, NOPE:] for h in hs], axis=1)
        wo_c = np.concatenate([Wo_hd[h] for h in hs], axis=0)
        l2_per_core.append(dict(
            Wqb=np.ascontiguousarray(wqb_c).astype(NF16),
            Wkn=np.ascontiguousarray(wkn_c).astype(NF16),
            Wv=np.ascontiguousarray(wv_c).astype(NF16),
            Wo=np.ascontiguousarray(wo_c).astype(NF16),
        ))

    return dict(xT=xT, cosW=cosW, sinW=sinW,
                Wqa=Wqa.astype(NF16), Wkva=Wkva.astype(NF16),
                perm128=_perm_rope_T(128),
                cosT2=np.concatenate([cosT, cosT], axis=0).astype(NF16),
                sinT2=np.concatenate([sinT, sinT], axis=0).astype(NF16),
                l2=l2_per_core)


def _get_programs():
    if "l1" not in _CACHE:
        _CACHE["l1"] = build_l1()
    if "l2" not in _CACHE:
        _CACHE["l2"] = build_l2()
    return _CACHE["l1"], _CACHE["l2"]


def kernel(x, mask, freqs_cis, Wqa, qln, Wqb, Wkva, kvln, Wkvb, Wo,
           _trace=False, _tmpdirs=None):
    p = _prep(x, freqs_cis, Wqa, qln, Wqb, Wkva, kvln, Wkvb, Wo)
    l1, l2 = _get_programs()

    in1 = []
    for c in range(N_CORES):
        sl = slice(c * S_LOC, (c + 1) * S_LOC)
        in1.append(dict(
            xT=np.ascontiguousarray(p["xT"][:, sl]),
            Wqa=p["Wqa"], Wkva=p["Wkva"],
            cosW=np.ascontiguousarray(p["cosW"][sl]),
            sinW=np.ascontiguousarray(p["sinW"][sl]),
        ))
    kw1 = {}
    if _trace:
        kw1 = dict(trace=True, tmpdir=(_tmpdirs or [None, None])[0])
    r1 = run_bass_kernel_spmd(l1, in1, core_ids=list(range(N_CORES)), **kw1)

    # host gather + transpose to feature-major for L2 (cheap numpy glue)
    tn_full = np.concatenate([r1.results[c]["tn"] for c in range(N_CORES)], axis=0)
    ckv_full = np.concatenate([r1.results[c]["ckv"] for c in range(N_CORES)], axis=0)
    tnT = np.ascontiguousarray(tn_full.T)                     # [1536, S]
    compT = np.ascontiguousarray(ckv_full[:, :KV_LORA].T)     # [512, S]
    kpeT = np.ascontiguousarray(ckv_full[:, KV_LORA:].T)      # [64, S]
    rqv = np.concatenate([r1.results[c]["rq"]
                          for c in range(N_CORES)]).astype(NF16)
    rkvv = np.concatenate([r1.results[c]["rkv"]
                           for c in range(N_CORES)]).astype(NF16)

    in2 = []
    for c in range(N_CORES):
        rq32 = rqv.astype(np.float32)[None, :]
        d = dict(tnT=tnT, compT=compT, kpeT=kpeT, rqv=rqv, rkvv=rkvv,
                 rkv32=rkvv.astype(np.float32),
                 cosT2=(p["cosT2"].astype(np.float32) * rq32).astype(NF16),
                 sinT2=(p["sinT2"].astype(np.float32) * rq32).astype(NF16),
                 permT2=p["perm128"])
        d.update(p["l2"][c])
        in2.append(d)
    kw2 = {}
    if _trace:
        kw2 = dict(trace=True, tmpdir=(_tmpdirs or [None, None])[1])
    r2 = run_bass_kernel_spmd(l2, in2, core_ids=list(range(N_CORES)), **kw2)

    acc = np.zeros((SEQ, D_MODEL), np.float64)
    for c in range(N_CORES):
        acc += r2.results[c]["out"].astype(np.float64)
    out = acc.astype(np.float32)[None]  # [1, S, D]

    kernel._last = (r1, r2)
    return out


# revision 32
# speedup vs baseline: 1.0021x; 1.0021x over previous
"""MLA attention (DeepSeek-style, LoRA Q/KV) on 8 Trainium2 NeuronCores.

Sharding (two SPMD launches, all tensor math in fp16 with fp32 PSUM):
  L1 (sequence-parallel, 256 tokens/core, token-major): each core computes
      t_raw  = x @ Wqa   -> rmsnorm -> tn  [256, 1536]
      ckv    = x @ Wkva  -> comp rmsnorm / kpe rope -> ckv [256, 576]
  Token-major layout makes the rmsnorm a per-partition (per-token) scale,
  applied by the Activation engine during the PSUM->SBUF copy; the sum of
  squares comes from an ACT Square pass with accum_out.  Weights are the
  moving matmul operand so the k-chunked DMA stream overlaps compute.
  Host gathers along tokens and transposes to feature-major (cheap numpy),
  then
  L2 (tensor-parallel, 2 heads/core): q/k/v LoRA-B projections, rope(q),
  scores^T = k @ q^T, biased exp (exp(s*scale - 2), bias cancels in the
  softmax ratio), softmax denominator via sequential fp16 adds of the exp
  tiles on the Vector engine + one Pool-engine partition_all_reduce (no
  TensorE work), attn_out^T = v @ exp^T, per-head normalize, output
  projection with this core's Wo row-slice.  Host sums the 8 partials.
"""

import math
from contextlib import ExitStack

import numpy as np

import concourse.mybir as mybir
import concourse.tile as tile
from concourse import bacc, bass_isa
from concourse.bass_utils import run_bass_kernel_spmd

F16 = mybir.dt.float16
F32 = mybir.dt.float32
AF = mybir.ActivationFunctionType
NF16 = np.float16

D_MODEL = 2048
NH = 16
Q_LORA = 1536
KV_LORA = 512
ROPE = 64
NOPE = 128
VDIM = 128
QHD = NOPE + ROPE  # 192
SEQ = 2048
N_CORES = 8
S_LOC = SEQ // N_CORES  # 256 tokens per core in L1
EPS = 1e-6
SCALE = 1.0 / math.sqrt(128.0)  # 1/sqrt(HEAD_DIM), as in the reference
EXP_BIAS = -2.0  # exp(s*SCALE + b): constant bias cancels in softmax

_CACHE = {}


def _perm_rope_T(n):
    """lhsT for P @ v where (P@v)[2i] = -v[2i+1], (P@v)[2i+1] = v[2i]."""
    P = np.zeros((n, n), np.float32)
    for i in range(n // 2):
        P[2 * i, 2 * i + 1] = -1.0
        P[2 * i + 1, 2 * i] = 1.0
    return np.ascontiguousarray(P.T).astype(NF16)


# --------------------------------------------------------------------------
# Launch 1: sequence-sharded LoRA-A projections + norms + k_pe rope
# (token-major: out[token, feature], weights are the moving operand)
# --------------------------------------------------------------------------

def build_l1():
    nc = bacc.Bacc("TRN2", target_bir_lowering=False, debug=False,
                   enable_asserts=True, num_devices=N_CORES)
    KD = D_MODEL // 128   # 16 contraction chunks

    xT = nc.dram_tensor("xT", [D_MODEL, S_LOC], F16, kind="ExternalInput").ap()
    Wqa = nc.dram_tensor("Wqa", [D_MODEL, Q_LORA], F16, kind="ExternalInput").ap()
    Wkva = nc.dram_tensor("Wkva", [D_MODEL, 576], F16, kind="ExternalInput").ap()
    # token-major cos/sin, pairs repeated: [S_LOC, 64] f32
    cosW = nc.dram_tensor("cosW", [S_LOC, ROPE], F32, kind="ExternalInput").ap()
    sinW = nc.dram_tensor("sinW", [S_LOC, ROPE], F32, kind="ExternalInput").ap()

    tn = nc.dram_tensor("tn", [S_LOC, Q_LORA], F16, kind="ExternalOutput").ap()
    ckv = nc.dram_tensor("ckv", [S_LOC, 576], F16, kind="ExternalOutput").ap()
    rq = nc.dram_tensor("rq", [S_LOC], F32, kind="ExternalOutput").ap()
    rkv = nc.dram_tensor("rkv", [S_LOC], F32, kind="ExternalOutput").ap()

    with tile.TileContext(nc) as tc, ExitStack() as ctx:
        const = ctx.enter_context(tc.tile_pool(name="const", bufs=1))
        big = ctx.enter_context(tc.tile_pool(name="big", bufs=1))
        work = ctx.enter_context(tc.tile_pool(name="work", bufs=2))
        ps = ctx.enter_context(tc.tile_pool(name="ps", bufs=1, space="PSUM"))

        # ---- DMA, one consumption-ordered queue: xT and Wqa column-0
        # k-chunks interleaved (Q phase runs first), then Wqa c1/c2, then
        # Wkva for the trailing KV phase.
        xT_r = xT.rearrange("(k p) s -> p k s", p=128)
        wqa_r = Wqa.rearrange("(k p) l -> p k l", p=128)
        wkva_r = Wkva.rearrange("(k p) l -> p k l", p=128)
        sb_xT = big.tile([128, KD, S_LOC], F16, tag="xT")
        sb_wqa = big.tile([128, KD, Q_LORA], F16, tag="wqa")
        sb_wkva = big.tile([128, KD, 576], F16, tag="wkva")
        for kg in range(0, KD, 4):
            nc.sync.dma_start(sb_xT[:, kg:kg + 4, :], xT_r[:, kg:kg + 4, :])
            nc.sync.dma_start(sb_wqa[:, kg:kg + 4, 0:512],
                              wqa_r[:, kg:kg + 4, 0:512])
        for kg in range(0, KD, 4):
            nc.sync.dma_start(sb_wqa[:, kg:kg + 4, 512:1024],
                              wqa_r[:, kg:kg + 4, 512:1024])
        for kg in range(0, KD, 8):
            nc.sync.dma_start(sb_wkva[:, kg:kg + 8, :], wkva_r[:, kg:kg + 8, :])
        for kg in range(0, KD, 4):
            nc.sync.dma_start(sb_wqa[:, kg:kg + 4, 1024:1536],
                              wqa_r[:, kg:kg + 4, 1024:1536])
        cos_r = cosW.rearrange("(t p) c -> p t c", p=128)
        sin_r = sinW.rearrange("(t p) c -> p t c", p=128)
        sb_cos = const.tile([128, 2, ROPE], F32, tag="cos")
        nc.scalar.dma_start(sb_cos[:], cos_r)
        sb_sin = const.tile([128, 2, ROPE], F32, tag="sin")
        nc.scalar.dma_start(sb_sin[:], sin_r)

        eps_t = const.tile([128, 1], F32, tag="eps")
        nc.vector.memset(eps_t[:], EPS)
        tn_r = tn.rearrange("(t p) f -> p t f", p=128)
        ckv_r = ckv.rearrange("(t p) f -> p t f", p=128)
        rq_r = rq.rearrange("(t p) -> p t", p=128)
        rkv_r = rkv.rearrange("(t p) -> p t", p=128)

        sq_dump = work.tile([128, 3 * 512], F16, tag="sqdump", bufs=2)

        def rms_r(sq_ap, dim, tag):
            """r = 1/sqrt(mean(sq)+eps) per partition (token)."""
            rstd = work.tile([128, 1], F32, tag=tag + "rs")
            nc.scalar.activation(rstd[:], sq_ap, AF.Sqrt, bias=eps_t[:],
                                 scale=1.0 / dim)
            r = work.tile([128, 1], F32, tag=tag + "r")
            nc.vector.reciprocal(r[:], rstd[:])
            return r

        # ---- Q phase (cols 0,1), then KV, then Q col 2.  Raw fp16
        # activations stream out as each chunk lands; only the tiny 1/rms
        # scale is computed here (normalization is folded into L2's copies).
        q_raw = big.tile([128, 2, 3 * 512], F16, tag="qraw")

        q_sq = [[None] * 3 for _ in range(2)]

        def q_tail(T):
            acc = q_sq[T][0][:]
            for c in (1, 2):
                nc.vector.tensor_add(acc, acc, q_sq[T][c][:])
            r = rms_r(acc, Q_LORA, f"q{T}")
            nc.sync.dma_start(rq_r[:, T:T + 1], r[:])

        q_cols = (0, 1)
        for c in q_cols:
            for T in range(2):
                acc = ps.tile([128, 512], F32, tag="q", bufs=4)
                for k in range(KD):
                    nc.tensor.matmul(acc[:], sb_xT[:, k, T * 128:(T + 1) * 128],
                                     sb_wqa[:, k, c * 512:(c + 1) * 512],
                                     start=(k == 0), stop=(k == KD - 1))
                nc.scalar.copy(q_raw[:, T, c * 512:(c + 1) * 512], acc[:])
                nc.sync.dma_start(tn_r[:, T, c * 512:(c + 1) * 512],
                                  q_raw[:, T, c * 512:(c + 1) * 512])
                sq = work.tile([128, 1], F32, tag=f"qsq{T}{c}")
                nc.scalar.activation(sq_dump[:, 0:512],
                                     q_raw[:, T, c * 512:(c + 1) * 512],
                                     AF.Square, accum_out=sq[:])
                q_sq[T][c] = sq
                if c == 2:
                    q_tail(T)


        # ---- KV phase (sandwiched between Q columns; tails hide under
        # the final Q column)
        ckv_sb = big.tile([128, 2, 576], F16, tag="ckvsb")
        for T in range(2):
            acc = ps.tile([128, 576], F32, tag="kv", bufs=2)
            for k in range(KD):
                nc.tensor.matmul(acc[:, 0:512], sb_xT[:, k, T * 128:(T + 1) * 128],
                                 sb_wkva[:, k, 0:512],
                                 start=(k == 0), stop=(k == KD - 1))
                nc.tensor.matmul(acc[:, 512:576], sb_xT[:, k, T * 128:(T + 1) * 128],
                                 sb_wkva[:, k, 512:576],
                                 start=(k == 0), stop=(k == KD - 1))
            nc.scalar.copy(ckv_sb[:, T, 0:512], acc[:, 0:512])
            sq = work.tile([128, 1], F32, tag="kvsq", bufs=2)
            nc.scalar.activation(sq_dump[:, 0:512], ckv_sb[:, T, 0:512],
                                 AF.Square, accum_out=sq[:])
            r = rms_r(sq[:], KV_LORA, f"kv{T}")
            nc.sync.dma_start(rkv_r[:, T:T + 1], r[:])
            # kpe rope: free-dim pair swap + cos/sin combine (DVE, tiny)
            sw = work.tile([128, ROPE], F32, tag="sw", bufs=2)
            nc.vector.tensor_scalar_mul(sw[:, 0:ROPE:2], acc[:, 513:576:2], -1.0)
            nc.vector.tensor_copy(sw[:, 1:ROPE:2], acc[:, 512:576:2])
            m1 = work.tile([128, ROPE], F32, tag="m1", bufs=2)
            nc.vector.tensor_mul(m1[:], acc[:, 512:576], sb_cos[:, T, :])
            m2 = work.tile([128, ROPE], F32, tag="m2", bufs=2)
            nc.vector.tensor_mul(m2[:], sw[:], sb_sin[:, T, :])
            nc.vector.tensor_add(ckv_sb[:, T, 512:576], m1[:], m2[:])
            nc.sync.dma_start(ckv_r[:, T, :], ckv_sb[:, T, :])

        q_cols = (2,)
        for c in q_cols:
            for T in range(2):
                acc = ps.tile([128, 512], F32, tag="q", bufs=4)
                for k in range(KD):
                    nc.tensor.matmul(acc[:], sb_xT[:, k, T * 128:(T + 1) * 128],
                                     sb_wqa[:, k, c * 512:(c + 1) * 512],
                                     start=(k == 0), stop=(k == KD - 1))
                nc.scalar.copy(q_raw[:, T, c * 512:(c + 1) * 512], acc[:])
                nc.sync.dma_start(tn_r[:, T, c * 512:(c + 1) * 512],
                                  q_raw[:, T, c * 512:(c + 1) * 512])
                sq = work.tile([128, 1], F32, tag=f"qsq{T}{c}")
                nc.scalar.activation(sq_dump[:, 0:512],
                                     q_raw[:, T, c * 512:(c + 1) * 512],
                                     AF.Square, accum_out=sq[:])
                q_sq[T][c] = sq
                if c == 2:
                    q_tail(T)

    nc.compile()
    return nc


# --------------------------------------------------------------------------
# Launch 2: head-sharded attention (2 heads per core)
# --------------------------------------------------------------------------

def build_l2():
    nc = bacc.Bacc("TRN2", target_bir_lowering=False, debug=False,
                   enable_asserts=True, num_devices=N_CORES)
    KQ = Q_LORA // 128    # 12
    KKV = KV_LORA // 128  # 4
    ST = SEQ // 128       # 16 key tiles
    SB = 1024             # query block
    NSB = SEQ // SB       # 2
    PRE_V = 2             # v-groups emitted right after kn (pre-paced)
    FILL_SCHED = [2, 1, 1, 1, 1, 0]  # v-groups per tnT k-pair in paced q

    tnT = nc.dram_tensor("tnT", [Q_LORA, SEQ], F16, kind="ExternalInput").ap()
    compT = nc.dram_tensor("compT", [KV_LORA, SEQ], F16, kind="ExternalInput").ap()
    kpeT = nc.dram_tensor("kpeT", [ROPE, SEQ], F16, kind="ExternalInput").ap()
    # Wqb cols reordered [h0 nope | h1 nope | h0 rope | h1 rope], qln folded
    Wqb = nc.dram_tensor("Wqb", [Q_LORA, 2 * QHD], F16, kind="ExternalInput").ap()
    Wkn = nc.dram_tensor("Wkn", [KV_LORA, 2 * NOPE], F16, kind="ExternalInput").ap()
    Wv = nc.dram_tensor("Wv", [KV_LORA, 2 * VDIM], F16, kind="ExternalInput").ap()
    Wo = nc.dram_tensor("Wo", [2 * VDIM, D_MODEL], F16, kind="ExternalInput").ap()
    rqv = nc.dram_tensor("rqv", [SEQ], F16, kind="ExternalInput").ap()
    rkvv = nc.dram_tensor("rkvv", [SEQ], F16, kind="ExternalInput").ap()
    rkv32 = nc.dram_tensor("rkv32", [SEQ], F32, kind="ExternalInput").ap()
    cosT2 = nc.dram_tensor("cosT2", [128, SEQ], F16, kind="ExternalInput").ap()
    sinT2 = nc.dram_tensor("sinT2", [128, SEQ], F16, kind="ExternalInput").ap()
    permT2 = nc.dram_tensor("permT2", [128, 128], F16, kind="ExternalInput").ap()

    out = nc.dram_tensor("out", [SEQ, D_MODEL], F16, kind="ExternalOutput").ap()

    with tile.TileContext(nc) as tc, ExitStack() as ctx:
        const = ctx.enter_context(tc.tile_pool(name="const", bufs=1))
        big = ctx.enter_context(tc.tile_pool(name="big", bufs=1))
        tmp1 = ctx.enter_context(tc.tile_pool(name="tmp1", bufs=1))
        work = ctx.enter_context(tc.tile_pool(name="work", bufs=2))
        exp_pool = ctx.enter_context(tc.tile_pool(name="expp", bufs=2))
        psum = ctx.enter_context(tc.tile_pool(name="psum", bufs=1, space="PSUM"))

        # DMA in consumption order.  Small weights on the ACT HWDGE queue,
        # big activation streams on the SP queue.
        sb_wkn = big.tile([128, KKV, 2 * NOPE], F16, tag="wkn")
        nc.sync.dma_start(sb_wkn[:], Wkn.rearrange("(k p) n -> p k n", p=128))
        sb_rkv1 = const.tile([1, SEQ], F16, tag="rkv1")
        nc.scalar.dma_start(sb_rkv1[:], rkvv.rearrange("(o s) -> o s", o=1))
        sb_rq1 = const.tile([1, SEQ], F16, tag="rq1")
        nc.scalar.dma_start(sb_rq1[:], rqv.rearrange("(o s) -> o s", o=1))
        # token-tiled layout for the per-partition v scaling (ACT needs f32)
        sb_rkv2 = const.tile([128, ST], F32, tag="rkv2")
        nc.scalar.dma_start(sb_rkv2[:], rkv32.rearrange("(t p) -> p t", p=128))
        rkv_bc = big.tile([128, SEQ], F16, tag="rkvbc")
        nc.gpsimd.partition_broadcast(rkv_bc[:], sb_rkv1[:])
        rq_bc = big.tile([128, SEQ], F16, tag="rqbc")
        nc.gpsimd.partition_broadcast(rq_bc[:], sb_rq1[:])
        sb_compT = big.tile([128, KKV, SEQ], F16, tag="compT")
        compT_r = compT.rearrange("(k p) s -> p k s", p=128)
        # column-major chunks so kn's n-th group only waits on chunk n
        for n in range(4):
            nc.sync.dma_start(sb_compT[:, :, n * 512:(n + 1) * 512],
                              compT_r[:, :, n * 512:(n + 1) * 512])
        sb_wv = big.tile([128, KKV, 2 * VDIM], F16, tag="wv")
        nc.sync.dma_start(sb_wv[:], Wv.rearrange("(k p) n -> p k n", p=128))
        sb_wqb = big.tile([128, KQ, 2 * QHD], F16, tag="wqb")
        nc.sync.dma_start(sb_wqb[:], Wqb.rearrange("(k p) n -> p k n", p=128))
        sb_tnT = big.tile([128, KQ, SEQ], F16, tag="tnT")
        tnT_r = tnT.rearrange("(k p) s -> p k s", p=128)
        sb_perm2 = const.tile([128, 128], F16, tag="perm2")
        sb_cos2 = const.tile([128, SEQ], F16, tag="cos2")
        sb_sin2 = const.tile([128, SEQ], F16, tag="sin2")
        # one strictly consumption-ordered queue: tnT chunks interleave with
        # the rope constants so both are resident when first needed
        nc.sync.dma_start(sb_tnT[:, 0:2, :], tnT_r[:, 0:2, :])
        nc.sync.dma_start(sb_perm2[:], permT2)
        nc.sync.dma_start(sb_tnT[:, 2:4, :], tnT_r[:, 2:4, :])
        nc.sync.dma_start(sb_cos2[:], cosT2)
        nc.sync.dma_start(sb_tnT[:, 4:6, :], tnT_r[:, 4:6, :])
        nc.sync.dma_start(sb_sin2[:], sinT2)
        for k in range(6, KQ, 2):
            nc.sync.dma_start(sb_tnT[:, k:k + 2, :], tnT_r[:, k:k + 2, :])
        sb_kpe = big.tile([ROPE, SEQ], F16, tag="kpe")
        nc.sync.dma_start(sb_kpe[:], kpeT)
        sb_wo = big.tile([128, 2, D_MODEL], F16, tag="wo")
        nc.sync.dma_start(sb_wo[:], Wo.rearrange("(k p) n -> p k n", p=128))

        ebias_t = const.tile([128, 1], F32, tag="ebias")
        nc.vector.memset(ebias_t[:], EXP_BIAS)

        # ---- k_nope^T per head, n-major so it is paced by the compT stream
        k_nope = [big.tile([128, SEQ], F16, tag=f"kn{h}", name=f"kn{h}")
                  for h in range(2)]
        for n in range(SEQ // 512):
            for h in range(2):
                acc = psum.tile([128, 512], F32, tag="sc", bufs=4, name="kacc")
                for k in range(KKV):
                    nc.tensor.matmul(acc[:], sb_wkn[:, k, h * 128:(h + 1) * 128],
                                     sb_compT[:, k, n * 512:(n + 1) * 512],
                                     start=(k == 0), stop=(k == KKV - 1))
                nc.vector.tensor_mul(k_nope[h][:, n * 512:(n + 1) * 512],
                                     acc[:], rkv_bc[:, n * 512:(n + 1) * 512])

        # ---- v natural [key, vd] per head, 4 key-tiles per PSUM tile.
        # Emitted as filler thunks inside the tnT-DMA-paced part of the q
        # projection (v depends only on compT + Wv, which land early).
        v_nat = [big.tile([128, ST, VDIM], F16, tag=f"v{h}", name=f"vn{h}")
                 for h in range(2)]

        def v_group(h, g):
            def go():
                acc = psum.tile([128, 512], F32, tag="sc", bufs=4, name="vacc")
                for i in range(4):
                    t = g * 4 + i
                    for k in range(KKV):
                        nc.tensor.matmul(acc[:, i * 128:(i + 1) * 128],
                                         sb_compT[:, k, t * 128:(t + 1) * 128],
                                         sb_wv[:, k, h * VDIM:(h + 1) * VDIM],
                                         start=(k == 0), stop=(k == KKV - 1))
                for i in range(4):
                    t = g * 4 + i
                    if i % 2 == 0:
                        nc.scalar.activation(v_nat[h][:, t, :],
                                             acc[:, i * 128:(i + 1) * 128],
                                             AF.Copy, scale=sb_rkv2[:, t:t + 1])
                    else:
                        nc.vector.tensor_scalar_mul(v_nat[h][:, t, :],
                                                    acc[:, i * 128:(i + 1) * 128],
                                                    sb_rkv2[:, t:t + 1])
            return go

        fillers = [v_group(h, g) for g in range(ST // 4) for h in range(2)]

        # ---- q^T = Wqb^T @ tn^T : m-tiles [h0 nope, h1 nope, (h0|h1) rope]
        q_nope = [big.tile([128, SEQ], F16, tag=f"qn{h}", name=f"qn{h}")
                  for h in range(2)]
        qpe_raw = tmp1.tile([128, SEQ], F16, tag="qpe_raw")
        qpe2 = big.tile([128, SEQ], F16, tag="qpe2")
        qswap = tmp1.tile([128, SEQ], F16, tag="qswap")

        def q_dst(m):
            return q_nope[m] if m < 2 else qpe_raw

        def rope_chunk(n):
            sl = slice(n * 512, (n + 1) * 512)
            sw = psum.tile([128, 512], F32, tag="sc", bufs=4, name="swps")
            nc.tensor.matmul(sw[:], sb_perm2[:], qpe_raw[:, sl],
                             start=True, stop=True)
            nc.scalar.copy(qswap[:, sl], sw[:])
            nc.vector.tensor_mul(qpe2[:, sl], qpe_raw[:, sl], sb_cos2[:, sl])
            nc.vector.tensor_mul(qswap[:, sl], qswap[:, sl], sb_sin2[:, sl])
            nc.vector.tensor_add(qpe2[:, sl], qpe2[:, sl], qswap[:, sl])

        # every column is a k-pair-interleaved m-triple so PE blocks only on
        # the genuinely-missing tnT chunk; v-group fillers absorb the rest of
        # the DMA-paced slack in the first wave
        for _ in range(PRE_V):
            fillers.pop(0)()
        for n in range(SEQ // 512):
            sl = slice(n * 512, (n + 1) * 512)
            accs = [psum.tile([128, 512], F32, tag=("sc" if m < 2 else "av"),
                              bufs=(4 if m < 2 else 2), name=f"qacc{n}{m}")
                    for m in range(3)]
            for kp in range(KQ // 2):
                for m in range(3):
                    for k in (2 * kp, 2 * kp + 1):
                        nc.tensor.matmul(accs[m][:],
                                         sb_wqb[:, k, m * 128:(m + 1) * 128],
                                         sb_tnT[:, k, sl],
                                         start=(k == 0), stop=(k == KQ - 1))
                if n == 0 and fillers:
                    for _ in range(FILL_SCHED[kp]):
                        if fillers:
                            fillers.pop(0)()
            for m in range(2):
                nc.vector.tensor_mul(q_dst(m)[:, sl], accs[m][:], rq_bc[:, sl])
            nc.scalar.copy(qpe_raw[:, sl], accs[2][:])
            if n > 0:
                rope_chunk(n - 1)
            if n == 0:
                while fillers:
                    fillers.pop(0)()
        rope_chunk(SEQ // 512 - 1)

        # h1 rope rows to a base-0 tile so matmul operands stay aligned
        qpe_h1 = big.tile([ROPE, SEQ], F16, tag="qpeh1")
        nc.gpsimd.dma_start(qpe_h1[:], qpe2[ROPE:128, :])

        def qpe_of(h):
            return qpe2[0:ROPE, :] if h == 0 else qpe_h1[:, :]

        # ---- attention per query block of SB, per head (exp streamed per
        # 512-wide chunk; denominator on DVE+Pool, no TensorE work)
        def attention_pass(sb_i, h):
            s0 = sb_i * SB
            av_ps = psum.tile([128, SB], F32, tag="av", bufs=2, name="av_ps")
            e_acc = work.tile([128, SB], F16, tag="eacc", bufs=2)

            def av_mm(t, n2, e):
                nc.tensor.matmul(av_ps[:, n2 * 512:(n2 + 1) * 512],
                                 v_nat[h][:, t, :], e[:],
                                 start=(t == 0), stop=(t == ST - 1))

            pending = []
            for t in range(ST):
                for n2 in range(2):
                    sl = slice(s0 + n2 * 512, s0 + (n2 + 1) * 512)
                    psl = slice(n2 * 512, (n2 + 1) * 512)
                    sc = psum.tile([128, 512], F32, tag="sc", bufs=4, name="sc")
                    nc.tensor.matmul(sc[:],
                                     k_nope[h][:, t * 128:(t + 1) * 128],
                                     q_nope[h][:, sl], start=True, stop=False)
                    nc.tensor.matmul(sc[:],
                                     sb_kpe[:, t * 128:(t + 1) * 128],
                                     qpe_of(h)[:, sl],
                                     start=False, stop=True)
                    expT = exp_pool.tile([128, 512], F16, tag="expT", bufs=6)
                    nc.scalar.activation(expT[:], sc[:], AF.Exp, bias=ebias_t[:],
                                         scale=SCALE)
                    if t == 0:
                        nc.vector.tensor_copy(e_acc[:, psl], expT[:])
                    else:
                        nc.vector.tensor_add(e_acc[:, psl], e_acc[:, psl], expT[:])
                    pending.append((t, n2, expT))
                    if len(pending) > 3:
                        av_mm(*pending.pop(0))
            for p_ in pending:
                av_mm(*p_)
            # denominator: one Pool partition_all_reduce, then per-chunk
            # reciprocal+scale so oproj can start on the first chunk early
            den_b = work.tile([128, SB], F32, tag="denb", bufs=2)
            nc.gpsimd.partition_all_reduce(den_b[:], e_acc[:], 128,
                                           bass_isa.ReduceOp.add)
            att = work.tile([128, SB], F16, tag=f"att{h}", name=f"att{h}")
            for n2 in range(2):
                psl = slice(n2 * 512, (n2 + 1) * 512)
                den_r = work.tile([128, 512], F32, tag="denr", bufs=2)
                nc.vector.reciprocal(den_r[:], den_b[:, psl])
                nc.vector.tensor_mul(att[:, psl], av_ps[:, psl], den_r[:])
            return att

        def oproj(sb_i, att_n, last=False):
            s0 = sb_i * SB
            for ms in range(SB // 128):
                o = work.tile([128, D_MODEL], F16, tag="osb", bufs=3)
                rows = slice(s0 + ms * 128, s0 + (ms + 1) * 128)
                chunked = last and ms == SB // 128 - 1
                for n in range(D_MODEL // 512):
                    cols = slice(n * 512, (n + 1) * 512)
                    acc = psum.tile([128, 512], F32, tag="sc", bufs=4,
                                    name="oacc")
                    for h in range(2):
                        nc.tensor.matmul(acc[:],
                                         att_n[h][:, ms * 128:(ms + 1) * 128],
                                         sb_wo[:, h, n * 512:(n + 1) * 512],
                                         start=(h == 0), stop=(h == 1))
                    if n % 2 == 0:
                        nc.scalar.copy(o[:, cols], acc[:])
                    else:
                        nc.vector.tensor_copy(o[:, cols], acc[:])
                    if chunked and n % 2 == 1:
                        half = slice((n - 1) * 512, (n + 1) * 512)
                        nc.sync.dma_start(out[rows, half], o[:, half])
                if not chunked:
                    nc.sync.dma_start(out[rows, :], o[:])

        # all passes first, then both output projections: the last
        # normalize chain hides under O0's matmuls instead of stalling PE
        a00 = attention_pass(0, 0)
        a01 = attention_pass(0, 1)
        a10 = attention_pass(1, 0)
        a11 = attention_pass(1, 1)
        oproj(0, [a00, a01])
        oproj(1, [a10, a11], last=True)

    nc.compile()
    return nc


# --------------------------------------------------------------------------
# Host orchestration
# --------------------------------------------------------------------------

def _prep(x, freqs_cis, Wqa, qln, Wqb, Wkva, kvln, Wkvb, Wo):
    """Host-side sharding prep (cheap numpy reshapes/casts only)."""
    xT = np.ascontiguousarray(x[0].T).astype(NF16)           # [D, S]
    cos = freqs_cis[..., 0].astype(np.float32)               # [S, 32]
    sin = freqs_cis[..., 1].astype(np.float32)
    cosW = np.repeat(cos, 2, axis=1)                         # [S, 64] tok-major
    sinW = np.repeat(sin, 2, axis=1)
    cosT = np.ascontiguousarray(cosW.T)                      # [64, S]
    sinT = np.ascontiguousarray(sinW.T)

    Wqb_f = Wqb * qln[:, None]
    Wkvb_f = Wkvb * kvln[:, None]
    Wqb_hd = Wqb_f.reshape(Q_LORA, NH, QHD)
    Wkvb_hd = Wkvb_f.reshape(KV_LORA, NH, NOPE + VDIM)
    Wo_hd = Wo.reshape(NH, VDIM, D_MODEL)
    l2_per_core = []
    for c in range(N_CORES):
        hs = [2 * c, 2 * c + 1]
        wqb_c = np.concatenate(
            [Wqb_hd[:, hs[0], :NOPE], Wqb_hd[:, hs[1], :NOPE],
             Wqb_hd[:, hs[0], NOPE:], Wqb_hd[:, hs[1], NOPE:]], axis=1)
        wkn_c = np.concatenate([Wkvb_hd[:, h, :NOPE] for h in hs], axis=1)
        wv_c = np.concatenate([Wkvb_hd[:, h, NOPE:] for h in hs], axis=1)
        wo_c = np.concatenate([Wo_hd[h] for h in hs], axis=0)
        l2_per_core.append(dict(
            Wqb=np.ascontiguousarray(wqb_c).astype(NF16),
            Wkn=np.ascontiguousarray(wkn_c).astype(NF16),
            Wv=np.ascontiguousarray(wv_c).astype(NF16),
            Wo=np.ascontiguousarray(wo_c).astype(NF16),
        ))

    return dict(xT=xT, cosW=cosW, sinW=sinW,
                Wqa=Wqa.astype(NF16), Wkva=Wkva.astype(NF16),
                perm128=_perm_rope_T(128),
                cosT2=np.concatenate([cosT, cosT], axis=0).astype(NF16),
                sinT2=np.concatenate([sinT, sinT], axis=0).astype(NF16),
                l2=l2_per_core)


def _get_programs():
    if "l1" not in _CACHE:
        _CACHE["l1"] = build_l1()
    if "l2" not in _CACHE:
        _CACHE["l2"] = build_l2()
    return _CACHE["l1"], _CACHE["l2"]


def kernel(x, mask, freqs_cis, Wqa, qln, Wqb, Wkva, kvln, Wkvb, Wo,
           _trace=False, _tmpdirs=None):
    p = _prep(x, freqs_cis, Wqa, qln, Wqb, Wkva, kvln, Wkvb, Wo)
    l1, l2 = _get_programs()

    in1 = []
    for c in range(N_CORES):
        sl = slice(c * S_LOC, (c + 1) * S_LOC)
        in1.append(dict(
            xT=np.ascontiguousarray(p["xT"][:, sl]),
            Wqa=p["Wqa"], Wkva=p["Wkva"],
            cosW=np.ascontiguousarray(p["cosW"][sl]),
            sinW=np.ascontiguousarray(p["sinW"][sl]),
        ))
    kw1 = {}
    if _trace:
        kw1 = dict(trace=True, tmpdir=(_tmpdirs or [None, None])[0])
    r1 = run_bass_kernel_spmd(l1, in1, core_ids=list(range(N_CORES)), **kw1)

    # host gather + transpose to feature-major for L2 (cheap numpy glue)
    tn_full = np.concatenate([r1.results[c]["tn"] for c in range(N_CORES)], axis=0)
    ckv_full = np.concatenate([r1.results[c]["ckv"] for c in range(N_CORES)], axis=0)
    tnT = np.ascontiguousarray(tn_full.T)                     # [1536, S]
    compT = np.ascontiguousarray(ckv_full[:, :KV_LORA].T)     # [512, S]
    kpeT = np.ascontiguousarray(ckv_full[:, KV_LORA:].T)      # [64, S]
    rqv = np.concatenate([r1.results[c]["rq"]
                          for c in range(N_CORES)]).astype(NF16)
    rkvv = np.concatenate([r1.results[c]["rkv"]
                           for c in range(N_CORES)]).astype(NF16)

    in2 = []
    for c in range(N_CORES):
        rq32 = rqv.astype(np.float32)[None, :]
        d = dict(tnT=tnT, compT=compT, kpeT=kpeT, rqv=rqv, rkvv=rkvv,
                 rkv32=rkvv.astype(np.float32),
                 cosT2=(p["cosT2"].astype(np.float32) * rq32).astype(NF16),
                 sinT2=(p["sinT2"].astype(np.float32) * rq32).astype(NF16),
                 permT2=p["perm128"])
        d.update(p["l2"][c])
        in2.append(d)
    kw2 = {}
    if _trace:
        kw2 = dict(trace=True, tmpdir=(_tmpdirs or [None, None])[1])
    r2 = run_bass_kernel_spmd(l2, in2, core_ids=list(range(N_CORES)), **kw2)

    acc = np.zeros((SEQ, D_MODEL), np.float64)
    for c in range(N_CORES):
        acc += r2.results[c]["out"].astype(np.float64)
    out = acc.astype(np.float32)[None]  # [1, S, D]

    kernel._last = (r1, r2)
    return out
